# revision 8
# baseline (speedup 1.0000x reference)
"""Causal attention (B=4, S=4096, D=64, fp32) on 8 Trainium2 NeuronCores.

Sharding: core = 2*b + s handles batch b (4 batches x 2 cores). Within a
batch, the 4096 q rows form 8 chunks of 512; chunk c needs k-tiles
0..4c+3 (causal). Core s=0 takes chunks {1,3,5,7} (k-tile counts
{8,16,24,32}), core s=1 takes chunks {0,2,4,6} (counts {4,12,20,28})
padded up to the same {8,16,24,32} so all 8 cores run one identical SPMD
program; pad k-tiles are masked via a 65th contraction row (-8192 bias ->
exp underflows to exactly 0).

Layout: scores are computed transposed, S^T[k,q] = K Q^T, with the
contraction dim d on SBUF partitions, so softmax normalization can be
deferred (a ones-column appended to V accumulates the row sums during the
P^T V matmul) and P^T feeds the PV matmul with no transposes. Diagonal
128x128 triangles are zeroed with affine_select at fixed program
positions (tail k-tiles are fed from per-chunk "slab" inputs that the
host orders as [full/pad x4, diag x4]).
"""

import numpy as np

import concourse.bass as bass  # noqa: F401  (keeps engine classes registered)
import concourse.mybir as mybir
from concourse import bacc
from concourse.tile import TileContext
from concourse.masks import make_identity
from concourse.bass_utils import run_bass_kernel_spmd

B, S, D = 4, 4096, 64
NCORES = 8
SLOT_C = (8, 16, 24, 32)  # k-tiles per chunk slot (uniform across cores)
NEG = -8192.0
F32 = mybir.dt.float32
F32R = mybir.dt.float32r
F16 = mybir.dt.float16

_prog_cache = None


def _build_program():
    global _prog_cache
    if _prog_cache is not None:
        return _prog_cache

    nc = bacc.Bacc("TRN2", target_bir_lowering=False, debug=False)
    qt_d = nc.declare_dram_parameter("qt", [65, 2048], F16, isOutput=False)
    ktm_d = nc.declare_dram_parameter("ktm", [64, 3072], F16, isOutput=False)
    kts_d = nc.declare_dram_parameter("kts", [65, 4096], F16, isOutput=False)
    vm_d = nc.declare_dram_parameter("vm", [128, 1560], F16, isOutput=False)
    vs_d = nc.declare_dram_parameter("vs", [128, 2080], F16, isOutput=False)
    o_d = nc.declare_dram_parameter("o", [2048, 64], F32, isOutput=True)
    EXP = mybir.ActivationFunctionType.Exp

    with TileContext(nc) as tc:
        with (
            tc.tile_pool(name="cons", bufs=1) as cons,
            tc.tile_pool(name="data", bufs=1) as data,
            tc.tile_pool(name="pp", bufs=3) as pp,
            tc.tile_pool(name="ep", bufs=2) as ep,
            tc.tile_pool(name="ps_sc", bufs=2, space="PSUM") as ps_sc,
            tc.tile_pool(name="ps_acc", bufs=1, space="PSUM") as ps_acc,
            tc.tile_pool(name="ps_t", bufs=1, space="PSUM") as ps_t,
        ):
            ident = cons.tile([128, 128], F32)
            make_identity(nc, ident[:])

            warm = cons.tile([128, 512], F16)
            nc.gpsimd.memset(warm[:], 0.0)
            for w in range(16):
                wp = ps_sc.tile([128, 1536], F32, tag="sc")
                nc.tensor.matmul(
                    wp[:, 0:512], warm[:, 0:128], warm[:], start=True, stop=True
                )

            qt = data.tile([65, 2048], F16)
            kts = data.tile([65, 4096], F16)
            vs = data.tile([128, 2080], F16)
            ktm = data.tile([64, 3072], F16)
            vm = data.tile([128, 1560], F16)

            def dma_slot(m):
                nc.sync.dma_start(
                    out=qt[:, 512 * m : 512 * (m + 1)],
                    in_=qt_d[:, 512 * m : 512 * (m + 1)],
                )
                nc.sync.dma_start(
                    out=kts[:, 1024 * m : 1024 * (m + 1)],
                    in_=kts_d[:, 1024 * m : 1024 * (m + 1)],
                )
                nc.sync.dma_start(
                    out=vs[:, 520 * m : 520 * (m + 1)],
                    in_=vs_d[:, 520 * m : 520 * (m + 1)],
                )

            def dma_main(g2):
                nc.sync.dma_start(
                    out=ktm[:, 768 * g2 : 768 * (g2 + 1)],
                    in_=ktm_d[:, 768 * g2 : 768 * (g2 + 1)],
                )
                nc.sync.dma_start(
                    out=vm[:, 390 * g2 : 390 * (g2 + 1)],
                    in_=vm_d[:, 390 * g2 : 390 * (g2 + 1)],
                )

            # ordered by first use: chunk0 is slab-only; chunk m needs ktm
            # tiles 0..C_m-9 before its slab
            dma_slot(0)
            dma_main(0)
            dma_main(1)
            dma_slot(1)
            dma_main(2)
            dma_slot(2)
            dma_main(3)
            dma_slot(3)

            for m in range(4):
                C = SLOT_C[m]
                q_sl = slice(512 * m, 512 * (m + 1))
                acc = ps_acc.tile([65, 512], F32, tag="acc")

                def emit_pv(pt, gang, C=C, m=m, acc=acc):
                    for d_, t in enumerate(gang):
                        ptile = pt[:, 512 * d_ : 512 * (d_ + 1)]
                        if t >= C - 4:
                            g = t - (C - 4)
                            nc.gpsimd.affine_select(
                                out=ptile,
                                in_=ptile,
                                compare_op=mybir.AluOpType.is_ge,
                                fill=0.0,
                                base=-128 * g,
                                pattern=[[1, 512]],
                                channel_multiplier=-1,
                            )
                        if t <= C - 9:
                            vt = vm[:, 65 * t : 65 * (t + 1)]
                        else:
                            p = t - (C - 8)
                            vt = vs[:, 520 * m + 65 * p : 520 * m + 65 * (p + 1)]
                        nc.tensor.matmul(
                            acc[:],
                            vt,
                            ptile,
                            start=(t == 0),
                            stop=(t == C - 1),
                        )

                pending = None
                gangs = [
                    list(range(t0, min(t0 + 3, C))) for t0 in range(0, C, 3)
                ]
                for gang in gangs:
                    sc = ps_sc.tile([128, 1536], F32, tag="sc")
                    for d_, t in enumerate(gang):
                        if t <= C - 9:
                            lhsT = ktm[:, 128 * t : 128 * (t + 1)]
                            rhs = qt[0:64, q_sl]
                        else:
                            p = t - (C - 8)
                            lhsT = kts[
                                :, 1024 * m + 128 * p : 1024 * m + 128 * (p + 1)
                            ]
                            rhs = qt[0:65, q_sl]
                        nc.tensor.matmul(
                            sc[:, 512 * d_ : 512 * (d_ + 1)],
                            lhsT,
                            rhs,
                            start=True,
                            stop=True,
                        )
                    pt = pp.tile([128, 1536], F16, tag="pt")
                    w = 512 * len(gang)
                    nc.scalar.activation(pt[:, :w], sc[:, :w], EXP, scale=0.125)
                    if pending is not None:
                        emit_pv(*pending)
                    pending = (pt, gang)
                emit_pv(*pending)

                osb = ep.tile([65, 512], F32, tag="osb")
                nc.vector.tensor_copy(osb[:], acc[:])
                oo = ep.tile([128, 256], F32, tag="oo")
                for j in range(4):
                    tp = ps_t.tile([128, 65], F32, tag="tp")
                    nc.tensor.transpose(
                        tp[:], osb[:, 128 * j : 128 * (j + 1)], ident[0:65, 0:65]
                    )
                    rec = ep.tile([128, 1], F32, tag="rec", bufs=8)
                    nc.vector.reciprocal(rec[:], tp[:, 64:65])
                    nc.vector.tensor_scalar_mul(
                        oo[:, 64 * j : 64 * (j + 1)], tp[:, 0:64], rec[:]
                    )
                nc.sync.dma_start(
                    out=o_d[512 * m : 512 * (m + 1), :].rearrange(
                        "(j p) d -> p j d", j=4
                    ),
                    in_=oo[:].rearrange("p (j d) -> p j d", j=4),
                )

    nc.compile()
    _prog_cache = nc
    return nc


def _prep_core_inputs(core, query, key, value):
    b, s = divmod(core, 2)
    qt = np.zeros((65, 2048), np.float16)
    qt[64, :] = 1.0
    kts = np.zeros((65, 4096), np.float16)
    vs = np.zeros((128, 2080), np.float16)
    ktm = np.ascontiguousarray(key[b, :3072, :].T.astype(np.float16))
    vaug = np.ones((S, 65), np.float16)
    vaug[:, :64] = value[b]
    vm = np.ascontiguousarray(
        vaug[: 24 * 128].reshape(24, 128, 65).transpose(1, 0, 2).reshape(128, 24 * 65)
    )
    for m in range(4):
        C = SLOT_C[m]
        c = 2 * m + 1 if s == 0 else 2 * m
        n = 4 * (c + 1)  # genuine k-tiles of this chunk
        qt[:64, 512 * m : 512 * (m + 1)] = query[b, 512 * c : 512 * (c + 1), :].T
        for p in range(8):
            col = slice(1024 * m + 128 * p, 1024 * m + 128 * (p + 1))
            vcol = slice(520 * m + 65 * p, 520 * m + 65 * (p + 1))
            if s == 0:
                t = C - 8 + p
            elif p < 4:
                kts[64, col] = NEG
                continue
            else:
                t = n - 8 + p  # p=4..7 -> diag tiles n-4..n-1
            kts[:64, col] = key[b, 128 * t : 128 * (t + 1), :].T
            vs[:, vcol] = vaug[128 * t : 128 * (t + 1), :]
    return {"qt": qt, "ktm": ktm, "kts": kts, "vm": vm, "vs": vs}


def run(query, key, value, trace=False, tmpdir=None):
    nc = _build_program()
    in_maps = [_prep_core_inputs(c, query, key, value) for c in range(NCORES)]
    res = run_bass_kernel_spmd(
        nc, in_maps, list(range(NCORES)), trace=trace, tmpdir=tmpdir
    )
    out = np.zeros((B, S, D), np.float32)
    for core in range(NCORES):
        b, s = divmod(core, 2)
        o = res.results[core]["o"]
        for m in range(4):
            c = 2 * m + 1 if s == 0 else 2 * m
            out[b, 512 * c : 512 * (c + 1), :] = o[512 * m : 512 * (m + 1), :]
    return out, res


def kernel(query, key, value):
    query = np.ascontiguousarray(np.asarray(query, dtype=np.float32))
    key = np.ascontiguousarray(np.asarray(key, dtype=np.float32))
    value = np.ascontiguousarray(np.asarray(value, dtype=np.float32))
    out, _ = run(query, key, value)
    return out


# revision 10
# speedup vs baseline: 1.1656x; 1.1656x over previous
"""Causal attention (B=4, S=4096, D=64, fp32) on 8 Trainium2 NeuronCores.

Sharding: two SPMD programs dispatched concurrently on disjoint device
sets. Within a batch, the 4096 q rows form 8 chunks of 512 columns;
chunk c needs k-tiles 0..4c+3 (causal). Program A (cores 0-3, one batch
each) takes chunks {1,3,5,7} (k-tile counts {8,16,24,32}); program B
(cores 4-7) takes chunks {0,2,4,6} (counts {4,12,20,28}). Both sum to 72
tile-passes per core - perfectly balanced.

Layout: scores are computed transposed, S^T[k,q] = K Q^T, with the
contraction dim d on SBUF partitions, so softmax normalization can be
deferred (a ones-column appended to V accumulates the row sums during
the P^T V matmul) and P^T feeds the PV matmul with no transposes. The
last up-to-8 k-tiles of each chunk come from per-chunk "slab" inputs;
the final 4 are the diagonal tiles, whose triangles are zeroed with
affine_select at fixed program positions. All matmul operands are fp16
(PE runs 1 cycle/row); accumulation stays fp32 in PSUM.
"""

import numpy as np

import jax
import concourse.bass as bass  # noqa: F401
import concourse.mybir as mybir
from concourse import bacc
from concourse import bass2jax
from concourse.tile import TileContext
from concourse.masks import make_identity

B, S, D = 4, 4096, 64
NCORES = 8
SLOT_A = (8, 16, 24, 32)  # program A: chunks {1,3,5,7} of a batch
SLOT_B = (4, 12, 20, 28)  # program B: chunks {0,2,4,6}
F32 = mybir.dt.float32
F16 = mybir.dt.float16

_cache = {}


def _chunk_index(slot_c, m):
    # chunk whose causal need equals slot_c[m]
    return slot_c[m] // 4 - 1


def _build_program(slot_c):
    n_shared = [max(c - 8, 0) for c in slot_c]
    n_slab = [min(c, 8) for c in slot_c]
    max_shared = max(n_shared)

    nc = bacc.Bacc("TRN2", target_bir_lowering=False, debug=False)
    qt_d = nc.declare_dram_parameter("qt", [65, 2048], F16, isOutput=False)
    ktm_d = nc.declare_dram_parameter(
        "ktm", [64, 128 * max_shared], F16, isOutput=False
    )
    kts_d = nc.declare_dram_parameter("kts", [65, 4096], F16, isOutput=False)
    vm_d = nc.declare_dram_parameter(
        "vm", [128, 65 * max_shared], F16, isOutput=False
    )
    vs_d = nc.declare_dram_parameter("vs", [128, 2080], F16, isOutput=False)
    o_d = nc.declare_dram_parameter("o", [2048, 64], F32, isOutput=True)
    EXP = mybir.ActivationFunctionType.Exp

    with TileContext(nc) as tc:
        with (
            tc.tile_pool(name="cons", bufs=1) as cons,
            tc.tile_pool(name="data", bufs=1) as data,
            tc.tile_pool(name="pp", bufs=3) as pp,
            tc.tile_pool(name="ep", bufs=2) as ep,
            tc.tile_pool(name="ps_sc", bufs=2, space="PSUM") as ps_sc,
            tc.tile_pool(name="ps_acc", bufs=2, space="PSUM") as ps_acc,
            tc.tile_pool(name="ps_t", bufs=2, space="PSUM") as ps_t,
        ):
            ident = cons.tile([128, 128], F32)
            make_identity(nc, ident[:])

            warm = cons.tile([128, 512], F16)
            nc.gpsimd.memset(warm[:], 0.0)
            for w in range(16):
                wp = ps_sc.tile([128, 1024], F32, tag="sc")
                nc.tensor.matmul(
                    wp[:, 0:512], warm[:, 0:128], warm[:], start=True, stop=True
                )

            qt = data.tile([65, 2048], F16)
            kts = data.tile([65, 4096], F16)
            vs = data.tile([128, 2080], F16)
            ktm = data.tile([64, 128 * max_shared], F16)
            vm = data.tile([128, 65 * max_shared], F16)

            def dma_slot(m):
                nc.sync.dma_start(
                    out=qt[:, 512 * m : 512 * (m + 1)],
                    in_=qt_d[:, 512 * m : 512 * (m + 1)],
                )
                nc.sync.dma_start(
                    out=kts[:, 1024 * m : 1024 * m + 128 * n_slab[m]],
                    in_=kts_d[:, 1024 * m : 1024 * m + 128 * n_slab[m]],
                )
                nc.sync.dma_start(
                    out=vs[:, 520 * m : 520 * m + 65 * n_slab[m]],
                    in_=vs_d[:, 520 * m : 520 * m + 65 * n_slab[m]],
                )

            def dma_main(lo, hi):  # shared k-tiles [lo, hi)
                if hi <= lo:
                    return
                nc.sync.dma_start(
                    out=ktm[:, 128 * lo : 128 * hi], in_=ktm_d[:, 128 * lo : 128 * hi]
                )
                nc.sync.dma_start(
                    out=vm[:, 65 * lo : 65 * hi], in_=vm_d[:, 65 * lo : 65 * hi]
                )

            # ordered by first use: each slot needs its shared tiles, then slab
            done = 0
            for m in range(4):
                mid = (done + n_shared[m]) // 2
                dma_main(done, mid)
                dma_main(mid, n_shared[m])
                done = max(done, n_shared[m])
                dma_slot(m)

            for m in range(4):
                C = slot_c[m]
                ns = n_shared[m]
                q_sl = slice(512 * m, 512 * (m + 1))
                acc = ps_acc.tile([65, 512], F32, tag="acc")

                def emit_pv(pt, gang, C=C, m=m, ns=ns, acc=acc):
                    for d_, t in enumerate(gang):
                        ptile = pt[:, 512 * d_ : 512 * (d_ + 1)]
                        if t >= C - 4:
                            g = t - (C - 4)
                            nc.gpsimd.affine_select(
                                out=ptile,
                                in_=ptile,
                                compare_op=mybir.AluOpType.is_ge,
                                fill=0.0,
                                base=-128 * g,
                                pattern=[[1, 512]],
                                channel_multiplier=-1,
                            )
                        if t < ns:
                            vt = vm[:, 65 * t : 65 * (t + 1)]
                        else:
                            p = t - ns
                            vt = vs[:, 520 * m + 65 * p : 520 * m + 65 * (p + 1)]
                        nc.tensor.matmul(
                            acc[:], vt, ptile, start=(t == 0), stop=(t == C - 1)
                        )

                pending = None
                for t0 in range(0, C, 2):
                    gang = list(range(t0, min(t0 + 2, C)))
                    sc = ps_sc.tile([128, 1024], F32, tag="sc")
                    for d_, t in enumerate(gang):
                        if t < ns:
                            lhsT = ktm[:, 128 * t : 128 * (t + 1)]
                            rhs = qt[0:64, q_sl]
                        else:
                            p = t - ns
                            lhsT = kts[
                                :, 1024 * m + 128 * p : 1024 * m + 128 * (p + 1)
                            ]
                            rhs = qt[0:65, q_sl]
                        nc.tensor.matmul(
                            sc[:, 512 * d_ : 512 * (d_ + 1)],
                            lhsT,
                            rhs,
                            start=True,
                            stop=True,
                        )
                    pt = pp.tile([128, 1024], F16, tag="pt")
                    w = 512 * len(gang)
                    nc.scalar.activation(pt[:, :w], sc[:, :w], EXP, scale=0.125)
                    if pending is not None:
                        emit_pv(*pending)
                    pending = (pt, gang)
                emit_pv(*pending)

                osb = ep.tile([65, 512], F32, tag="osb")
                nc.vector.tensor_copy(osb[:], acc[:])
                oo = ep.tile([128, 256], F32, tag="oo")
                for j in range(4):
                    tp = ps_t.tile([128, 65], F32, tag="tp")
                    nc.tensor.transpose(
                        tp[:], osb[:, 128 * j : 128 * (j + 1)], ident[0:65, 0:65]
                    )
                    rec = ep.tile([128, 1], F32, tag="rec", bufs=8)
                    nc.vector.reciprocal(rec[:], tp[:, 64:65])
                    nc.vector.tensor_scalar_mul(
                        oo[:, 64 * j : 64 * (j + 1)], tp[:, 0:64], rec[:]
                    )
                nc.sync.dma_start(
                    out=o_d[512 * m : 512 * (m + 1), :].rearrange(
                        "(j p) d -> p j d", j=4
                    ),
                    in_=oo[:].rearrange("p (j d) -> p j d", j=4),
                )

    nc.compile()
    return nc


def _prep_core_inputs(slot_c, b, query, key, value):
    n_shared = [max(c - 8, 0) for c in slot_c]
    n_slab = [min(c, 8) for c in slot_c]
    max_shared = max(n_shared)

    qt = np.zeros((65, 2048), np.float16)
    qt[64, :] = 1.0
    kts = np.zeros((65, 4096), np.float16)
    vs = np.zeros((128, 2080), np.float16)
    ktm = np.ascontiguousarray(key[b, : 128 * max_shared, :].T.astype(np.float16))
    vaug = np.ones((S, 65), np.float16)
    vaug[:, :64] = value[b]
    vm = np.ascontiguousarray(
        vaug[: 128 * max_shared]
        .reshape(max_shared, 128, 65)
        .transpose(1, 0, 2)
        .reshape(128, 65 * max_shared)
    )
    for m in range(4):
        c = _chunk_index(slot_c, m)
        qt[:64, 512 * m : 512 * (m + 1)] = query[b, 512 * c : 512 * (c + 1), :].T
        for p in range(n_slab[m]):
            t = n_shared[m] + p
            col = slice(1024 * m + 128 * p, 1024 * m + 128 * (p + 1))
            vcol = slice(520 * m + 65 * p, 520 * m + 65 * (p + 1))
            kts[:64, col] = key[b, 128 * t : 128 * (t + 1), :].T
            vs[:, vcol] = vaug[128 * t : 128 * (t + 1), :]
    return {"qt": qt, "ktm": ktm, "kts": kts, "vm": vm, "vs": vs}


def _make_runner(nc, devices):
    """Vendored multi-core run_bass_via_pjrt with an explicit device set,
    split into an async dispatch and a blocking unpack."""
    from jax.sharding import Mesh, PartitionSpec

    bass2jax.install_neuronx_cc_hook()
    n = len(devices)
    partition_name = nc.partition_id_tensor.name if nc.partition_id_tensor else None
    in_names, out_names, out_avals, zero_outs = [], [], [], []
    for alloc in nc.m.functions[0].allocations:
        if not isinstance(alloc, mybir.MemoryLocationSet):
            continue
        name = alloc.memorylocations[0].name
        if alloc.kind == "ExternalInput":
            if name != partition_name:
                in_names.append(name)
        elif alloc.kind == "ExternalOutput":
            out_names.append(name)
            shape = tuple(alloc.tensor_shape)
            dtype = mybir.dt.np(alloc.dtype)
            out_avals.append(jax.core.ShapedArray(shape, dtype))
            zero_outs.append(np.zeros(shape, dtype))
    n_params = len(in_names)
    all_in = list(in_names) + list(out_names)
    if partition_name is not None:
        all_in.append(partition_name)
    all_in = tuple(all_in)
    donate = tuple(range(n_params, n_params + len(out_names)))

    def _body(*args):
        operands = list(args)
        if partition_name is not None:
            operands.append(bass2jax.partition_id_tensor())
        outs = bass2jax._bass_exec_p.bind(
            *operands,
            out_avals=tuple(out_avals),
            in_names=all_in,
            out_names=tuple(out_names),
            lowering_input_output_aliases=(),
            sim_require_finite=True,
            sim_require_nnan=True,
            nc=nc,
        )
        return tuple(outs)

    mesh = Mesh(np.asarray(devices), ("core",))
    in_specs = (PartitionSpec("core"),) * (n_params + len(out_names))
    out_specs = (PartitionSpec("core"),) * len(out_names)
    sharded = jax.jit(
        jax.shard_map(
            _body, mesh=mesh, in_specs=in_specs, out_specs=out_specs, check_vma=False
        ),
        donate_argnums=donate,
        keep_unused=True,
    )

    def dispatch(in_maps):
        concat_in = [
            np.concatenate([np.asarray(in_maps[c][nm]) for c in range(n)], axis=0)
            for nm in in_names
        ]
        concat_zeros = [
            np.zeros((n * z.shape[0], *z.shape[1:]), z.dtype) for z in zero_outs
        ]
        return sharded(*concat_in, *concat_zeros)

    def unpack(out_arrs):
        return [
            {
                nm: np.asarray(out_arrs[i]).reshape(n, *out_avals[i].shape)[c]
                for i, nm in enumerate(out_names)
            }
            for c in range(n)
        ]

    return dispatch, unpack


def _get_engine():
    if "engine" not in _cache:
        devs = jax.devices()
        ncA = _build_program(SLOT_A)
        ncB = _build_program(SLOT_B)
        dispA, unpackA = _make_runner(ncA, devs[0:4])
        dispB, unpackB = _make_runner(ncB, devs[4:8])
        _cache["engine"] = (dispA, unpackA, dispB, unpackB)
        _cache["ncs"] = (ncA, ncB)
    return _cache["engine"]


def run(query, key, value):
    dispA, unpackA, dispB, unpackB = _get_engine()
    mapsA = [_prep_core_inputs(SLOT_A, b, query, key, value) for b in range(4)]
    mapsB = [_prep_core_inputs(SLOT_B, b, query, key, value) for b in range(4)]
    outA = dispA(mapsA)
    outB = dispB(mapsB)
    resA = unpackA(outA)
    resB = unpackB(outB)

    out = np.zeros((B, S, D), np.float32)
    for b in range(4):
        for slot_c, res in ((SLOT_A, resA[b]), (SLOT_B, resB[b])):
            o = res["o"]
            for m in range(4):
                c = _chunk_index(slot_c, m)
                out[b, 512 * c : 512 * (c + 1), :] = o[512 * m : 512 * (m + 1), :]
    return out


def kernel(query, key, value):
    query = np.ascontiguousarray(np.asarray(query, dtype=np.float32))
    key = np.ascontiguousarray(np.asarray(key, dtype=np.float32))
    value = np.ascontiguousarray(np.asarray(value, dtype=np.float32))
    return run(query, key, value)


# revision 11
# speedup vs baseline: 1.2572x; 1.0786x over previous
"""Causal attention (B=4, S=4096, D=64, fp32) on 8 Trainium2 NeuronCores.

Sharding: two SPMD programs dispatched concurrently on disjoint device
sets. Within a batch, the 4096 q rows form 8 chunks of 512 columns;
chunk c needs k-tiles 0..4c+3 (causal). Program A (cores 0-3, one batch
each) takes chunks {1,3,5,7} (k-tile counts {8,16,24,32}); program B
(cores 4-7) takes chunks {0,2,4,6} (counts {4,12,20,28}). Both sum to 72
tile-passes per core - perfectly balanced.

Layout: scores are computed transposed, S^T[k,q] = K Q^T, with the
contraction dim d on SBUF partitions, so softmax normalization can be
deferred (a ones-column appended to V accumulates the row sums during
the P^T V matmul) and P^T feeds the PV matmul with no transposes. The
last up-to-8 k-tiles of each chunk come from per-chunk "slab" inputs;
the final 4 are the diagonal tiles, whose triangles are zeroed with
affine_select at fixed program positions. All matmul operands are fp16
(PE runs 1 cycle/row); accumulation stays fp32 in PSUM.
"""

import numpy as np

import jax
import concourse.bass as bass  # noqa: F401
import concourse.mybir as mybir
from concourse import bacc
from concourse import bass2jax
from concourse.tile import TileContext
from concourse.masks import make_identity

B, S, D = 4, 4096, 64
NCORES = 8
SLOT_A = (4, 8, 28, 32)  # program A: chunks {0,1,6,7} of a batch (72 tiles)
SLOT_B = (12, 16, 20, 24)  # program B: chunks {2,3,4,5} (72 tiles)
F32 = mybir.dt.float32
F16 = mybir.dt.float16

_cache = {}


def _chunk_index(slot_c, m):
    # chunk whose causal need equals slot_c[m]
    return slot_c[m] // 4 - 1


def _build_program(slot_c):
    n_shared = [max(c - 8, 0) for c in slot_c]
    n_slab = [min(c, 8) for c in slot_c]
    max_shared = max(n_shared)

    nc = bacc.Bacc("TRN2", target_bir_lowering=False, debug=False)
    qt_d = nc.declare_dram_parameter("qt", [65, 2048], F16, isOutput=False)
    ktm_d = nc.declare_dram_parameter(
        "ktm", [64, 128 * max_shared], F16, isOutput=False
    )
    kts_d = nc.declare_dram_parameter("kts", [65, 4096], F16, isOutput=False)
    vm_d = nc.declare_dram_parameter(
        "vm", [128, 65 * max_shared], F16, isOutput=False
    )
    vs_d = nc.declare_dram_parameter("vs", [128, 2080], F16, isOutput=False)
    o_d = nc.declare_dram_parameter("o", [2048, 64], F32, isOutput=True)
    EXP = mybir.ActivationFunctionType.Exp

    with TileContext(nc) as tc:
        with (
            tc.tile_pool(name="cons", bufs=1) as cons,
            tc.tile_pool(name="data", bufs=1) as data,
            tc.tile_pool(name="pp", bufs=3) as pp,
            tc.tile_pool(name="ep", bufs=2) as ep,
            tc.tile_pool(name="ps_sc", bufs=2, space="PSUM") as ps_sc,
            tc.tile_pool(name="ps_acc", bufs=2, space="PSUM") as ps_acc,
            tc.tile_pool(name="ps_t", bufs=2, space="PSUM") as ps_t,
        ):
            ident = cons.tile([128, 128], F32)
            make_identity(nc, ident[:])

            warm = cons.tile([128, 512], F16)
            nc.gpsimd.memset(warm[:], 0.0)
            for w in range(16):
                wp = ps_sc.tile([128, 1024], F32, tag="sc")
                nc.tensor.matmul(
                    wp[:, 0:512], warm[:, 0:128], warm[:], start=True, stop=True
                )

            qt = data.tile([65, 2048], F16)
            kts = data.tile([65, 4096], F16)
            vs = data.tile([128, 2080], F16)
            ktm = data.tile([64, 128 * max_shared], F16)
            vm = data.tile([128, 65 * max_shared], F16)

            def dma_slot(m):
                nc.sync.dma_start(
                    out=qt[:, 512 * m : 512 * (m + 1)],
                    in_=qt_d[:, 512 * m : 512 * (m + 1)],
                )
                nc.sync.dma_start(
                    out=kts[:, 1024 * m : 1024 * m + 128 * n_slab[m]],
                    in_=kts_d[:, 1024 * m : 1024 * m + 128 * n_slab[m]],
                )
                nc.sync.dma_start(
                    out=vs[:, 520 * m : 520 * m + 65 * n_slab[m]],
                    in_=vs_d[:, 520 * m : 520 * m + 65 * n_slab[m]],
                )

            def dma_main(lo, hi):  # shared k-tiles [lo, hi)
                if hi <= lo:
                    return
                nc.sync.dma_start(
                    out=ktm[:, 128 * lo : 128 * hi], in_=ktm_d[:, 128 * lo : 128 * hi]
                )
                nc.sync.dma_start(
                    out=vm[:, 65 * lo : 65 * hi], in_=vm_d[:, 65 * lo : 65 * hi]
                )

            # ordered by first use: each slot needs its shared tiles, then slab
            done = 0
            for m in range(4):
                mid = (done + n_shared[m]) // 2
                dma_main(done, mid)
                dma_main(mid, n_shared[m])
                done = max(done, n_shared[m])
                dma_slot(m)

            for m in range(4):
                C = slot_c[m]
                ns = n_shared[m]
                q_sl = slice(512 * m, 512 * (m + 1))
                acc = ps_acc.tile([65, 512], F32, tag="acc")

                def emit_pv(pt, gang, C=C, m=m, ns=ns, acc=acc):
                    for d_, t in enumerate(gang):
                        ptile = pt[:, 512 * d_ : 512 * (d_ + 1)]
                        if t >= C - 4:
                            g = t - (C - 4)
                            nc.gpsimd.affine_select(
                                out=ptile,
                                in_=ptile,
                                compare_op=mybir.AluOpType.is_ge,
                                fill=0.0,
                                base=-128 * g,
                                pattern=[[1, 512]],
                                channel_multiplier=-1,
                            )
                        if t < ns:
                            vt = vm[:, 65 * t : 65 * (t + 1)]
                        else:
                            p = t - ns
                            vt = vs[:, 520 * m + 65 * p : 520 * m + 65 * (p + 1)]
                        nc.tensor.matmul(
                            acc[:], vt, ptile, start=(t == 0), stop=(t == C - 1)
                        )

                pending = None
                for t0 in range(0, C, 2):
                    gang = list(range(t0, min(t0 + 2, C)))
                    sc = ps_sc.tile([128, 1024], F32, tag="sc")
                    for d_, t in enumerate(gang):
                        if t < ns:
                            lhsT = ktm[:, 128 * t : 128 * (t + 1)]
                            rhs = qt[0:64, q_sl]
                        else:
                            p = t - ns
                            lhsT = kts[
                                :, 1024 * m + 128 * p : 1024 * m + 128 * (p + 1)
                            ]
                            rhs = qt[0:65, q_sl]
                        nc.tensor.matmul(
                            sc[:, 512 * d_ : 512 * (d_ + 1)],
                            lhsT,
                            rhs,
                            start=True,
                            stop=True,
                        )
                    pt = pp.tile([128, 1024], F16, tag="pt")
                    w = 512 * len(gang)
                    nc.scalar.activation(pt[:, :w], sc[:, :w], EXP, scale=0.125)
                    if pending is not None:
                        emit_pv(*pending)
                    pending = (pt, gang)
                emit_pv(*pending)

                osb = ep.tile([65, 512], F32, tag="osb")
                nc.vector.tensor_copy(osb[:], acc[:])
                oo = ep.tile([128, 256], F32, tag="oo")
                for j in range(4):
                    tp = ps_t.tile([128, 65], F32, tag="tp")
                    nc.tensor.transpose(
                        tp[:], osb[:, 128 * j : 128 * (j + 1)], ident[0:65, 0:65]
                    )
                    rec = ep.tile([128, 1], F32, tag="rec", bufs=8)
                    nc.vector.reciprocal(rec[:], tp[:, 64:65])
                    nc.vector.tensor_scalar_mul(
                        oo[:, 64 * j : 64 * (j + 1)], tp[:, 0:64], rec[:]
                    )
                nc.sync.dma_start(
                    out=o_d[512 * m : 512 * (m + 1), :].rearrange(
                        "(j p) d -> p j d", j=4
                    ),
                    in_=oo[:].rearrange("p (j d) -> p j d", j=4),
                )

    nc.compile()
    return nc


def _prep_core_inputs(slot_c, b, query, key, value):
    n_shared = [max(c - 8, 0) for c in slot_c]
    n_slab = [min(c, 8) for c in slot_c]
    max_shared = max(n_shared)

    qt = np.zeros((65, 2048), np.float16)
    qt[64, :] = 1.0
    kts = np.zeros((65, 4096), np.float16)
    vs = np.zeros((128, 2080), np.float16)
    ktm = np.ascontiguousarray(key[b, : 128 * max_shared, :].T.astype(np.float16))
    vaug = np.ones((S, 65), np.float16)
    vaug[:, :64] = value[b]
    vm = np.ascontiguousarray(
        vaug[: 128 * max_shared]
        .reshape(max_shared, 128, 65)
        .transpose(1, 0, 2)
        .reshape(128, 65 * max_shared)
    )
    for m in range(4):
        c = _chunk_index(slot_c, m)
        qt[:64, 512 * m : 512 * (m + 1)] = query[b, 512 * c : 512 * (c + 1), :].T
        for p in range(n_slab[m]):
            t = n_shared[m] + p
            col = slice(1024 * m + 128 * p, 1024 * m + 128 * (p + 1))
            vcol = slice(520 * m + 65 * p, 520 * m + 65 * (p + 1))
            kts[:64, col] = key[b, 128 * t : 128 * (t + 1), :].T
            vs[:, vcol] = vaug[128 * t : 128 * (t + 1), :]
    return {"qt": qt, "ktm": ktm, "kts": kts, "vm": vm, "vs": vs}


def _make_runner(nc, devices):
    """Vendored multi-core run_bass_via_pjrt with an explicit device set,
    split into an async dispatch and a blocking unpack."""
    from jax.sharding import Mesh, PartitionSpec

    bass2jax.install_neuronx_cc_hook()
    n = len(devices)
    partition_name = nc.partition_id_tensor.name if nc.partition_id_tensor else None
    in_names, out_names, out_avals, zero_outs = [], [], [], []
    for alloc in nc.m.functions[0].allocations:
        if not isinstance(alloc, mybir.MemoryLocationSet):
            continue
        name = alloc.memorylocations[0].name
        if alloc.kind == "ExternalInput":
            if name != partition_name:
                in_names.append(name)
        elif alloc.kind == "ExternalOutput":
            out_names.append(name)
            shape = tuple(alloc.tensor_shape)
            dtype = mybir.dt.np(alloc.dtype)
            out_avals.append(jax.core.ShapedArray(shape, dtype))
            zero_outs.append(np.zeros(shape, dtype))
    n_params = len(in_names)
    all_in = list(in_names) + list(out_names)
    if partition_name is not None:
        all_in.append(partition_name)
    all_in = tuple(all_in)
    donate = tuple(range(n_params, n_params + len(out_names)))

    def _body(*args):
        operands = list(args)
        if partition_name is not None:
            operands.append(bass2jax.partition_id_tensor())
        outs = bass2jax._bass_exec_p.bind(
            *operands,
            out_avals=tuple(out_avals),
            in_names=all_in,
            out_names=tuple(out_names),
            lowering_input_output_aliases=(),
            sim_require_finite=True,
            sim_require_nnan=True,
            nc=nc,
        )
        return tuple(outs)

    mesh = Mesh(np.asarray(devices), ("core",))
    in_specs = (PartitionSpec("core"),) * (n_params + len(out_names))
    out_specs = (PartitionSpec("core"),) * len(out_names)
    sharded = jax.jit(
        jax.shard_map(
            _body, mesh=mesh, in_specs=in_specs, out_specs=out_specs, check_vma=False
        ),
        donate_argnums=donate,
        keep_unused=True,
    )

    def dispatch(in_maps):
        concat_in = [
            np.concatenate([np.asarray(in_maps[c][nm]) for c in range(n)], axis=0)
            for nm in in_names
        ]
        concat_zeros = [
            np.zeros((n * z.shape[0], *z.shape[1:]), z.dtype) for z in zero_outs
        ]
        return sharded(*concat_in, *concat_zeros)

    def unpack(out_arrs):
        return [
            {
                nm: np.asarray(out_arrs[i]).reshape(n, *out_avals[i].shape)[c]
                for i, nm in enumerate(out_names)
            }
            for c in range(n)
        ]

    return dispatch, unpack


def _get_engine():
    if "engine" not in _cache:
        devs = jax.devices()
        ncA = _build_program(SLOT_A)
        ncB = _build_program(SLOT_B)
        dispA, unpackA = _make_runner(ncA, devs[0:4])
        dispB, unpackB = _make_runner(ncB, devs[4:8])
        _cache["engine"] = (dispA, unpackA, dispB, unpackB)
        _cache["ncs"] = (ncA, ncB)
    return _cache["engine"]


def run(query, key, value):
    dispA, unpackA, dispB, unpackB = _get_engine()
    mapsA = [_prep_core_inputs(SLOT_A, b, query, key, value) for b in range(4)]
    mapsB = [_prep_core_inputs(SLOT_B, b, query, key, value) for b in range(4)]
    outA = dispA(mapsA)
    outB = dispB(mapsB)
    resA = unpackA(outA)
    resB = unpackB(outB)

    out = np.zeros((B, S, D), np.float32)
    for b in range(4):
        for slot_c, res in ((SLOT_A, resA[b]), (SLOT_B, resB[b])):
            o = res["o"]
            for m in range(4):
                c = _chunk_index(slot_c, m)
                out[b, 512 * c : 512 * (c + 1), :] = o[512 * m : 512 * (m + 1), :]
    return out


def kernel(query, key, value):
    query = np.ascontiguousarray(np.asarray(query, dtype=np.float32))
    key = np.ascontiguousarray(np.asarray(key, dtype=np.float32))
    value = np.ascontiguousarray(np.asarray(value, dtype=np.float32))
    return run(query, key, value)
